# revision 4
# baseline (speedup 1.0000x reference)
"""Cross-attention block (q/k/v proj -> gated softmax attention -> out proj
-> residual + LayerNorm) on 8 Trainium2 NeuronCores.

Sharding: data-parallel over batch (B=4) x query-row halves (2) = 8 shards.
Each core handles one (b, m-half): computes full k/v projections for its
batch, attention for all 16 heads over its 512 query rows, output
projection, residual + LayerNorm. No collectives; host concatenates.

Layout strategy (all matmul operands live in SBUF as float32r):
  - Activations are pre-transposed on host to contraction-major layouts:
      Qt  = Q[b].T          [D, M_SH]   (lhs rhs for q-proj)
      KVt = KV[b].T         [D, N]
      W*t = W.T             [D_in, D_out]
  - q/k projections produce head-major (o-major) tiles directly:
      qT [o, m], kT [o, n]  -> exactly the lhsT/rhs layouts attention needs.
  - Scores are computed TRANSPOSED: S^T[n, m] = kT_slice.T @ qT_slice, so
    softmax's additive gate bias (per n) is a per-partition ACT bias and
    exp(S/8 + gate) is a single fused ACT op from PSUM. No row-max pass
    (scores are O(+-8); exp is fp32-safe).
  - v is produced in natural [n, dh] layout with a ones-column appended per
    head, so P^T-matmul accumulates both attn@v AND the softmax denominator
    in one pass: psum[65, m] per head. Normalisation is folded in after PV.
  - PV output [dh, m] is k-major: stacked heads form the o-proj lhsT with no
    transposes anywhere in the kernel.
"""
import numpy as np

import concourse.bass as bass
import concourse.mybir as mybir
import concourse.tile as tile
from concourse import bacc
from concourse.bass_utils import run_bass_kernel_spmd

F32 = mybir.dt.float32
F32R = mybir.dt.float32r
AFT = mybir.ActivationFunctionType

B, M, N, D = 4, 1024, 2048, 1024
H, DH = 16, 64
M_SH = M // 2          # query rows per core
G = 4                  # head groups
HPG = H // G           # heads per group
OG = HPG * DH          # 256 output cols per group
KT = D // 128          # 8 contraction subtiles
LN_EPS = 1e-5
SCALE = 1.0 / np.sqrt(DH)
N_CORES = 8

_CACHE = {}


def build_nc(reps=1):
    """Build the per-core Bass program.

    reps>1 emits the full kernel body `reps` times into one NEFF (same
    output, recomputed). Used only for timing: the slope between a
    reps=R and reps=1 dispatch isolates true on-device execution time
    from the constant host->device dispatch overhead of this setup.
    """
    nc = bacc.Bacc("TRN2", target_bir_lowering=False, debug=False)

    qt = nc.dram_tensor("qt", [D, M_SH], F32, kind="ExternalInput")
    qn = nc.dram_tensor("qn", [M_SH, D], F32, kind="ExternalInput")
    kvt = nc.dram_tensor("kvt", [D, N], F32, kind="ExternalInput")
    wqt = nc.dram_tensor("wqt", [D, D], F32, kind="ExternalInput")
    wkt = nc.dram_tensor("wkt", [D, D], F32, kind="ExternalInput")
    wvt = nc.dram_tensor("wvt", [D, D], F32, kind="ExternalInput")
    wot = nc.dram_tensor("wot", [D, D], F32, kind="ExternalInput")
    gate = nc.dram_tensor("gate", [N], F32, kind="ExternalInput")
    bq = nc.dram_tensor("bq", [D], F32, kind="ExternalInput")
    bk = nc.dram_tensor("bk", [D], F32, kind="ExternalInput")
    bv = nc.dram_tensor("bv", [D], F32, kind="ExternalInput")
    bo = nc.dram_tensor("bo", [D], F32, kind="ExternalInput")
    gamma = nc.dram_tensor("gamma", [D], F32, kind="ExternalInput")
    beta = nc.dram_tensor("beta", [D], F32, kind="ExternalInput")
    onesc = nc.dram_tensor("onesc", [DH], F32, kind="ExternalInput")
    out = nc.dram_tensor("out", [M_SH, D], F32, kind="ExternalOutput")

    # contraction-major DRAM views [128, KT, x]
    qt_v = qt.rearrange("(s p) m -> p s m", p=128)
    kvt_v = kvt.rearrange("(s p) n -> p s n", p=128)
    wqt_v = wqt.rearrange("(s p) o -> p s o", p=128)
    wkt_v = wkt.rearrange("(s p) o -> p s o", p=128)
    wvt_v = wvt.rearrange("(s p) o -> p s o", p=128)
    wot_v = wot.rearrange("(s p) o -> p s o", p=128)

    with tile.TileContext(nc) as tc:
        with tc.tile_pool(name="consts", bufs=1) as consts, \
             tc.tile_pool(name="stream", bufs=2) as stream, \
             tc.tile_pool(name="wg", bufs=3) as wgp, \
             tc.tile_pool(name="qtg", bufs=2) as qtgp, \
             tc.tile_pool(name="ktg", bufs=2) as ktgp, \
             tc.tile_pool(name="vbig", bufs=2) as vbigp, \
             tc.tile_pool(name="pt", bufs=2) as ptp, \
             tc.tile_pool(name="aot", bufs=1) as aotp, \
             tc.tile_pool(name="small", bufs=2) as small, \
             tc.tile_pool(name="outst", bufs=1) as outst, \
             tc.tile_pool(name="ps", bufs=2, space="PSUM") as psp, \
             tc.tile_pool(name="s2", bufs=2, space="PSUM") as s2p, \
             tc.tile_pool(name="pv", bufs=2, space="PSUM") as pvp:

            # ---- constants ----
            gate_sb = consts.tile([128, N // 128], F32, tag="gate")
            nc.sync.dma_start(gate_sb[:], gate.rearrange("(t p) -> p t", p=128))
            bq_sb = consts.tile([128, KT], F32, tag="bq")
            nc.sync.dma_start(bq_sb[:], bq.rearrange("(t p) -> p t", p=128))
            bk_sb = consts.tile([128, KT], F32, tag="bk")
            nc.sync.dma_start(bk_sb[:], bk.rearrange("(t p) -> p t", p=128))
            bv_b = consts.tile([128, D], F32, tag="bv")
            nc.sync.dma_start(bv_b[:], bv[None, :].to_broadcast((128, D)))
            bo_b = consts.tile([128, D], F32, tag="bo")
            nc.sync.dma_start(bo_b[:], bo[None, :].to_broadcast((128, D)))
            gamma_b = consts.tile([128, D], F32, tag="gamma")
            nc.sync.dma_start(gamma_b[:], gamma[None, :].to_broadcast((128, D)))
            beta_b = consts.tile([128, D], F32, tag="beta")
            nc.sync.dma_start(beta_b[:], beta[None, :].to_broadcast((128, D)))
            eps_sb = consts.tile([128, 1], F32, tag="eps")
            nc.vector.memset(eps_sb[:], LN_EPS)
            ones1 = consts.tile([1, DH], F32R, tag="ones1")
            nc.gpsimd.dma_start(ones1[:], onesc[None, :])

            for rep in range(reps):
                emit_rep(nc, rep, consts, stream, wgp, qtgp, ktgp, vbigp,
                         ptp, aotp, small, outst, psp, s2p, pvp,
                         qt_v, kvt_v, wqt_v, wkt_v, wvt_v, wot_v, qn, out,
                         onesc, gate_sb, bq_sb, bk_sb, bv_b, bo_b,
                         gamma_b, beta_b, eps_sb, ones1)

    nc.compile()
    return nc


def emit_rep(nc, rep, consts, stream, wgp, qtgp, ktgp, vbigp, ptp, aotp,
             small, outst, psp, s2p, pvp,
             qt_v, kvt_v, wqt_v, wkt_v, wvt_v, wot_v, qn, out,
             onesc, gate_sb, bq_sb, bk_sb, bv_b, bo_b, gamma_b, beta_b,
             eps_sb, ones1):
    """One full kernel body (q/k/v proj -> attention -> o-proj -> LN)."""
    R = f"r{rep}"

    # q activations, contraction-major, fp32r (split DMAs so the first
    # q-proj matmuls start earlier)
    qt_sb = consts.tile([128, KT, M_SH], F32R, tag="qt")
    for h in range(4):
        nc.gpsimd.dma_start(qt_sb[:, 2 * h:2 * h + 2],
                            qt_v[:, 2 * h:2 * h + 2])

    # o-proj accumulator input: stacked head outputs [o, m]
    aot = aotp.tile([128, KT, M_SH], F32R, tag="aot")

    gstate = {}

    def emit_group_setup(g):
        ob = g * OG
        wq_g = wgp.tile([128, KT, OG], F32R, tag="w", name=f"wq_{g}{R}")
        nc.gpsimd.dma_start(wq_g[:], wqt_v[:, :, ob:ob + OG])
        wk_g = wgp.tile([128, KT, OG], F32R, tag="w", name=f"wk_{g}{R}")
        nc.gpsimd.dma_start(wk_g[:], wkt_v[:, :, ob:ob + OG])
        wv_g = wgp.tile([128, KT, OG], F32R, tag="w", name=f"wv_{g}{R}")
        nc.gpsimd.dma_start(wv_g[:], wvt_v[:, :, ob:ob + OG])
        # q projection: qT_g[o_local, m]
        qT_g = qtgp.tile([128, 2, M_SH], F32R, tag="qtg",
                         name=f"qT_{g}{R}")
        for ot in range(2):
            ps = psp.tile([128, M_SH], F32, tag="mm", name=f"psq{g}{ot}{R}")
            for kt in range(KT):
                nc.tensor.matmul(
                    ps[:], wq_g[:, kt, ot * 128:(ot + 1) * 128],
                    qt_sb[:, kt], start=(kt == 0), stop=(kt == KT - 1))
            nc.vector.tensor_scalar_add(
                qT_g[:, ot], ps[:], bq_sb[:, 2 * g + ot, None])
        kT_g = ktgp.tile([128, 2, N], F32R, tag="ktg", name=f"kT_{g}{R}")
        v_big = vbigp.tile([128, N // 128, HPG, DH + 1], F32R, tag="v",
                           name=f"v_{g}{R}")
        nc.gpsimd.dma_start(
            v_big[:, :, :, DH],
            onesc.rearrange("(a b) -> a b", a=N // 128)[None]
            .to_broadcast((128, N // 128, HPG)))
        gstate[g] = (wk_g, wv_g, qT_g, kT_g, v_big)

    def emit_kv_chunk(g, ch):
        wk_g, wv_g, qT_g, kT_g, v_big = gstate[g]
        ob = g * OG
        ck = stream.tile([128, KT, 512], F32R, tag="ck",
                         name=f"ck_{g}_{ch}{R}")
        for h in range(2):
            nc.gpsimd.dma_start(
                ck[:, 4 * h:4 * h + 4],
                kvt_v[:, 4 * h:4 * h + 4, ch * 512:(ch + 1) * 512])
        for ot in range(2):
            ps = psp.tile([128, 512], F32, tag="mm", name=f"psk{g}{ch}{ot}{R}")
            for kt in range(KT):
                nc.tensor.matmul(
                    ps[:], wk_g[:, kt, ot * 128:(ot + 1) * 128],
                    ck[:, kt], start=(kt == 0), stop=(kt == KT - 1))
            nc.vector.tensor_scalar_add(
                kT_g[:, ot, ch * 512:(ch + 1) * 512], ps[:],
                bk_sb[:, 2 * g + ot, None])
        for ntl in range(4):
            nt = ch * 4 + ntl
            psv = psp.tile([128, 512], F32, tag="mm", name=f"psv{g}{nt}{R}")
            for kt in range(KT):
                nc.tensor.matmul(
                    psv[:, 0:OG],
                    ck[:, kt, ntl * 128:(ntl + 1) * 128],
                    wv_g[:, kt], start=(kt == 0), stop=(kt == KT - 1))
            nc.vector.tensor_add(
                out=v_big[:, nt, :, 0:DH],
                in0=psv[:, 0:OG].rearrange("p (j d) -> p j d", j=HPG),
                in1=bv_b[:, ob:ob + OG].rearrange(
                    "p (j d) -> p j d", j=HPG))

    def emit_attn_nt(g, wave, nt, pv_ps):
        # one head PAIR per wave iteration: both S^T matmuls write the
        # same two-bank psum so a single wide ACT does exp for both.
        _, _, qT_g, kT_g, v_big = gstate[g]
        j0, j1 = 2 * wave, 2 * wave + 1
        ps2 = s2p.tile([128, 2, M_SH], F32, tag="s2",
                       name=f"pss{g}{wave}{nt}{R}")
        for i, j in enumerate((j0, j1)):
            base, tl = (j % 2) * 64, j // 2
            nc.tensor.matmul(
                ps2[:, i],
                kT_g[base:base + 64, tl, nt * 128:(nt + 1) * 128],
                qT_g[base:base + 64, tl, :],
                start=True, stop=True)
        pt_t = ptp.tile([128, 2, M_SH], F32R, tag="pt",
                        name=f"pt{g}{wave}{nt}{R}")
        nc.scalar.activation(
            out=pt_t[:], in_=ps2[:], func=AFT.Exp,
            bias=gate_sb[:, nt, None], scale=SCALE)
        for i, j in enumerate((j0, j1)):
            nc.tensor.matmul(
                pv_ps[i][:], v_big[:, nt, j, :], pt_t[:, i],
                start=(nt == 0), stop=(nt == N // 128 - 1))

    def emit_tails(g, wave, pv_ps):
        # normalise by accumulated denominator row; pack into aot.
        # Drain the pv psum FIRST (recip of the denominator row + raw
        # copy) so its bank frees quickly for the next wave; the
        # broadcast+multiply then run off SBUF/another bank.
        for i, j in enumerate((2 * wave, 2 * wave + 1)):
            recip = small.tile([1, M_SH], F32R, tag="recip",
                               name=f"rc{g}{j}{R}")
            with nc.allow_low_precision(
                    reason="fp32r operand for PE broadcast matmul"):
                nc.vector.reciprocal(recip[:], pv_ps[i][DH:DH + 1, :])
            ao_raw = small.tile([DH, M_SH], F32, tag="ao_raw",
                                name=f"ar{g}{j}{R}")
            nc.scalar.activation(out=ao_raw[:], in_=pv_ps[i][0:DH, :],
                                 func=AFT.Copy, bias=0.0, scale=1.0)
            ps_b = psp.tile([128, M_SH], F32, tag="mm",
                            name=f"psb{g}{j}{R}")
            nc.tensor.matmul(ps_b[0:DH, :], ones1[:], recip[:],
                             start=True, stop=True)
            ao_t = small.tile([DH, M_SH], F32R, tag="aot_tmp",
                              name=f"ao{g}{j}{R}")
            nc.vector.tensor_mul(out=ao_t[:], in0=ps_b[0:DH, :],
                                 in1=ao_raw[:])
            pb = (j % 2) * 64
            nc.sync.dma_start(
                aot[pb:pb + DH, 2 * g + j // 2, :], ao_t[:])

    # software pipeline: attention of group g interleaves with the
    # k/v projection chunks (and setup) of group g+1, so the PE has
    # independent matmul work whenever it would stall on ACT exp.
    emit_group_setup(0)
    for ch in range(N // 512):
        emit_kv_chunk(0, ch)
    wo_c = []
    for g in range(G):
        if g + 1 < G:
            emit_group_setup(g + 1)
        else:
            # prefetch the o-proj weights into the (now idle) stream
            # slots while the last group's attention runs
            for oc in range(2):
                w = stream.tile([128, KT, 512], F32R, tag="ck",
                                name=f"wo_{oc}{R}")
                nc.gpsimd.dma_start(
                    w[:], wot_v[:, :, oc * 512:(oc + 1) * 512])
                wo_c.append(w)
        for wave in range(2):
            pv_ps = [pvp.tile([DH + 1, M_SH], F32, tag="pv",
                              name=f"pv_{g}_{wave}_{i}{R}")
                     for i in range(2)]
            for nt in range(N // 128):
                emit_attn_nt(g, wave, nt, pv_ps)
                if g + 1 < G and nt % 8 == 7:
                    emit_kv_chunk(g + 1, 2 * wave + nt // 8)
            emit_tails(g, wave, pv_ps)
        del gstate[g]

    # ---- output projection + bias + residual + LayerNorm ----
    for mt in range(M_SH // 128):
        x_t = outst.tile([128, D], F32, tag="x")
        qn_t = outst.tile([128, D], F32, tag="qn")
        nc.sync.dma_start(qn_t[:], qn[mt * 128:(mt + 1) * 128, :])
        for oc in range(2):
            ps = psp.tile([128, 512], F32, tag="mm")
            for kt in range(KT):
                nc.tensor.matmul(
                    ps[:], aot[:, kt, mt * 128:(mt + 1) * 128],
                    wo_c[oc][:, kt], start=(kt == 0), stop=(kt == KT - 1))
            nc.vector.tensor_add(out=x_t[:, oc * 512:(oc + 1) * 512],
                                 in0=ps[:],
                                 in1=bo_b[:, oc * 512:(oc + 1) * 512])
        nc.vector.tensor_add(out=x_t[:], in0=x_t[:], in1=qn_t[:])
        # LayerNorm over D=1024 (two bn_stats subgroups of 512)
        st = outst.tile([128, 2, 6], F32, tag="st")
        nc.vector.bn_stats(st[:, 0], x_t[:, 0:512])
        nc.vector.bn_stats(st[:, 1], x_t[:, 512:1024])
        mv = outst.tile([128, 2], F32, tag="mv")
        nc.vector.bn_aggr(mv[:], st[:])
        nm = outst.tile([128, 1], F32, tag="nm")
        nc.vector.tensor_scalar_mul(nm[:], mv[:, 0:1], -1.0)
        rstd = outst.tile([128, 1], F32, tag="rstd")
        nc.scalar.activation(out=rstd[:], in_=mv[:, 1:2],
                             func=AFT.Sqrt, bias=eps_sb[:], scale=1.0)
        nc.vector.reciprocal(rstd[:], rstd[:])
        nc.vector.tensor_scalar_add(x_t[:], x_t[:], nm[:])
        nc.vector.tensor_scalar_mul(x_t[:], x_t[:], rstd[:])
        nc.vector.tensor_mul(out=x_t[:], in0=x_t[:], in1=gamma_b[:])
        nc.vector.tensor_add(out=x_t[:], in0=x_t[:], in1=beta_b[:])
        nc.sync.dma_start(out[mt * 128:(mt + 1) * 128, :], x_t[:])


def make_in_maps(inputs):
    f = lambda x: np.ascontiguousarray(np.asarray(x, dtype=np.float32))
    Q, KV = f(inputs["Q"]), f(inputs["KV"])
    gate = f(inputs["log_gate_bias"])
    wqt = f(np.asarray(inputs["Wq"]).T)
    wkt = f(np.asarray(inputs["Wk"]).T)
    wvt = f(np.asarray(inputs["Wv"]).T)
    wot = f(np.asarray(inputs["Wo"]).T)
    shared = {
        "wqt": wqt, "wkt": wkt, "wvt": wvt, "wot": wot,
        "bq": f(inputs["bq"]), "bk": f(inputs["bk"]),
        "bv": f(inputs["bv"]), "bo": f(inputs["bo"]),
        "gamma": f(inputs["gamma"]), "beta": f(inputs["beta"]),
        "onesc": np.ones(DH, dtype=np.float32),
    }
    in_maps = []
    for c in range(N_CORES):
        b, mh = c // 2, c % 2
        qt_b = np.ascontiguousarray(Q[b].T[:, mh * M_SH:(mh + 1) * M_SH])
        in_maps.append({
            "qt": qt_b,
            "qn": np.ascontiguousarray(Q[b, mh * M_SH:(mh + 1) * M_SH, :]),
            "kvt": np.ascontiguousarray(KV[b].T),
            "gate": np.ascontiguousarray(gate[b]),
            **shared,
        })
    return in_maps


def assemble(results):
    out = np.empty((B, M, D), dtype=np.float32)
    for c in range(N_CORES):
        b, mh = c // 2, c % 2
        out[b, mh * M_SH:(mh + 1) * M_SH, :] = results[c]["out"]
    return out


def kernel(**inputs) -> np.ndarray:
    if "nc" not in _CACHE:
        _CACHE["nc"] = build_nc()
    nc = _CACHE["nc"]
    in_maps = make_in_maps(inputs)
    res = run_bass_kernel_spmd(nc, in_maps, core_ids=list(range(N_CORES)))
    return assemble(res.results)



# revision 8
# speedup vs baseline: 6.6862x; 6.6862x over previous
"""Cross-attention block (q/k/v proj -> gated softmax attention -> out proj
-> residual + LayerNorm) on 8 Trainium2 NeuronCores.

Sharding: data-parallel over batch (B=4) x query-row halves (2) = 8 shards.
Each core handles one (b, m-half): computes full k/v projections for its
batch, attention for all 16 heads over its 512 query rows, output
projection, residual + LayerNorm. No collectives; host concatenates.

Layout strategy:
  - All matmul operands are bf16 (activations + weights cast on host);
    every accumulation stays fp32 in PSUM, and softmax/LayerNorm math is
    fp32. Error stays ~1e-3 scale-relative (gate is 2e-2) while halving
    HBM/DMA traffic — the measured HW bottleneck.
  - KV^T is loaded into SBUF ONCE per execution (bf16, 4MB) and all four
    head-group k/v projections stream it from SBUF (the fp32 predecessor
    re-read it from HBM per group: 32MB/exec).
  - Activations are pre-transposed on host to contraction-major layouts:
      qt  = Q[b].T          [D, M_SH]
      kvt = KV[b].T         [D, N]
      w*t = W.T             [D_in, D_out]
  - q/k projections produce head-major (o-major) tiles directly:
      qT [o, m], kT [o, n]  -> exactly the lhsT/rhs layouts attention needs.
  - Scores are computed TRANSPOSED: S^T[n, m] = kT_slice.T @ qT_slice, so
    softmax's additive gate bias (per n) is a per-partition ACT bias and
    exp(S/8 + gate) is a single fused ACT op from PSUM. No row-max pass
    (scores are O(+-8); exp is fp32-safe).
  - v is produced in natural [n, dh] layout with a ones-column appended per
    head (memset, no DMA), so P^T-matmul accumulates both attn@v AND the
    softmax denominator in one pass: psum[65, m] per head. Normalisation is
    folded in after PV.
  - PV output [dh, m] is k-major: stacked heads form the o-proj lhsT with no
    transposes anywhere in the kernel.
"""
import numpy as np

import concourse.bass as bass
import concourse.mybir as mybir
import concourse.tile as tile
from concourse import bacc
from concourse.bass_utils import run_bass_kernel_spmd

F32 = mybir.dt.float32
F32R = mybir.dt.float32r
BF16 = mybir.dt.bfloat16
AFT = mybir.ActivationFunctionType

B, M, N, D = 4, 1024, 2048, 1024
H, DH = 16, 64
M_SH = M // 2          # query rows per core
G = 4                  # head groups
HPG = H // G           # heads per group
OG = HPG * DH          # 256 output cols per group
KT = D // 128          # 8 contraction subtiles
LN_EPS = 1e-5
SCALE = 1.0 / np.sqrt(DH)
N_CORES = 8

_CACHE = {}


def build_nc(reps=1, level=3):
    """Build the per-core Bass program.

    reps>1 emits the full kernel body `reps` times into one NEFF (same
    output, recomputed). Used only for timing: the slope between a
    reps=R and reps=1 dispatch isolates true on-device execution time
    from the constant host->device dispatch overhead of this setup.

    level (timing experiments only; <3 gives wrong results): 0 = input
    DMA loads only, 1 = +q/k/v projections, 2 = +attention, 3 = full.
    """
    nc = bacc.Bacc("TRN2", target_bir_lowering=False, debug=False)

    qt = nc.dram_tensor("qt", [D, M_SH], BF16, kind="ExternalInput")
    qn = nc.dram_tensor("qn", [M_SH, D], F32, kind="ExternalInput")
    kvt = nc.dram_tensor("kvt", [D, N], BF16, kind="ExternalInput")
    wqt = nc.dram_tensor("wqt", [D, D], BF16, kind="ExternalInput")
    wkt = nc.dram_tensor("wkt", [D, D], BF16, kind="ExternalInput")
    wvt = nc.dram_tensor("wvt", [D, D], BF16, kind="ExternalInput")
    wot = nc.dram_tensor("wot", [D, D], BF16, kind="ExternalInput")
    gate = nc.dram_tensor("gate", [N], F32, kind="ExternalInput")
    bq = nc.dram_tensor("bq", [D], F32, kind="ExternalInput")
    bk = nc.dram_tensor("bk", [D], F32, kind="ExternalInput")
    bv = nc.dram_tensor("bv", [D], F32, kind="ExternalInput")
    bo = nc.dram_tensor("bo", [D], F32, kind="ExternalInput")
    gamma = nc.dram_tensor("gamma", [D], F32, kind="ExternalInput")
    beta = nc.dram_tensor("beta", [D], F32, kind="ExternalInput")
    onesc = nc.dram_tensor("onesc", [DH], F32, kind="ExternalInput")
    onesb = nc.dram_tensor("onesb", [DH], BF16, kind="ExternalInput")
    out = nc.dram_tensor("out", [M_SH, D], F32, kind="ExternalOutput")

    # contraction-major DRAM views [128, KT, x]
    qt_v = qt.rearrange("(s p) m -> p s m", p=128)
    kvt_v = kvt.rearrange("(s p) n -> p s n", p=128)
    wqt_v = wqt.rearrange("(s p) o -> p s o", p=128)
    wkt_v = wkt.rearrange("(s p) o -> p s o", p=128)
    wvt_v = wvt.rearrange("(s p) o -> p s o", p=128)
    wot_v = wot.rearrange("(s p) o -> p s o", p=128)

    with tile.TileContext(nc) as tc:
        with tc.tile_pool(name="consts", bufs=1) as consts, \
             tc.tile_pool(name="ckall", bufs=1) as ckp, \
             tc.tile_pool(name="stream", bufs=2) as stream, \
             tc.tile_pool(name="wg", bufs=3) as wgp, \
             tc.tile_pool(name="qtg", bufs=2) as qtgp, \
             tc.tile_pool(name="ktg", bufs=2) as ktgp, \
             tc.tile_pool(name="vbig", bufs=2) as vbigp, \
             tc.tile_pool(name="pt", bufs=2) as ptp, \
             tc.tile_pool(name="aot", bufs=1) as aotp, \
             tc.tile_pool(name="small", bufs=2) as small, \
             tc.tile_pool(name="outst", bufs=1) as outst, \
             tc.tile_pool(name="ps", bufs=2, space="PSUM") as psp, \
             tc.tile_pool(name="s2", bufs=2, space="PSUM") as s2p, \
             tc.tile_pool(name="pv", bufs=2, space="PSUM") as pvp:

            # ---- constants ----
            gate_sb = consts.tile([128, N // 128], F32, tag="gate")
            nc.sync.dma_start(gate_sb[:], gate.rearrange("(t p) -> p t", p=128))
            bq_sb = consts.tile([128, KT], F32, tag="bq")
            nc.sync.dma_start(bq_sb[:], bq.rearrange("(t p) -> p t", p=128))
            bk_sb = consts.tile([128, KT], F32, tag="bk")
            nc.sync.dma_start(bk_sb[:], bk.rearrange("(t p) -> p t", p=128))
            bv_b = consts.tile([128, D], F32, tag="bv")
            nc.sync.dma_start(bv_b[:], bv[None, :].to_broadcast((128, D)))
            bo_b = consts.tile([128, D], F32, tag="bo")
            nc.sync.dma_start(bo_b[:], bo[None, :].to_broadcast((128, D)))
            gamma_b = consts.tile([128, D], F32, tag="gamma")
            nc.sync.dma_start(gamma_b[:], gamma[None, :].to_broadcast((128, D)))
            beta_b = consts.tile([128, D], F32, tag="beta")
            nc.sync.dma_start(beta_b[:], beta[None, :].to_broadcast((128, D)))
            eps_sb = consts.tile([128, 1], F32, tag="eps")
            nc.vector.memset(eps_sb[:], LN_EPS)
            ones1 = consts.tile([1, DH], F32R, tag="ones1")
            nc.gpsimd.dma_start(ones1[:], onesc[None, :])

            for rep in range(reps):
                emit_rep(nc, rep, level, consts, ckp, stream, wgp, qtgp, ktgp,
                         vbigp, ptp, aotp, small, outst, psp, s2p, pvp,
                         qt_v, kvt_v, wqt_v, wkt_v, wvt_v, wot_v, qn, out,
                         onesb, gate_sb, bq_sb, bk_sb, bv_b, bo_b,
                         gamma_b, beta_b, eps_sb, ones1)

    nc.compile()
    return nc


def emit_rep(nc, rep, level, consts, ckp, stream, wgp, qtgp, ktgp, vbigp,
             ptp, aotp, small, outst, psp, s2p, pvp,
             qt_v, kvt_v, wqt_v, wkt_v, wvt_v, wot_v, qn, out,
             onesb, gate_sb, bq_sb, bk_sb, bv_b, bo_b, gamma_b, beta_b,
             eps_sb, ones1):
    """One full kernel body (q/k/v proj -> attention -> o-proj -> LN)."""
    R = f"r{rep}"

    # q activations, contraction-major (split DMAs so the first q-proj
    # matmuls start earlier)
    qt_sb = consts.tile([128, KT, M_SH], BF16, tag="qt")
    for h in range(2):
        nc.gpsimd.dma_start(qt_sb[:, 4 * h:4 * h + 4],
                            qt_v[:, 4 * h:4 * h + 4])

    # KV^T resident in SBUF, loaded once (bf16, 4MB)
    ck_all = ckp.tile([128, KT, N], BF16, tag="ckall")
    for ch in range(4):
        nc.gpsimd.dma_start(ck_all[:, :, ch * 512:(ch + 1) * 512],
                            kvt_v[:, :, ch * 512:(ch + 1) * 512])

    # o-proj accumulator input: stacked head outputs [o, m]
    aot = aotp.tile([128, KT, M_SH], BF16, tag="aot")

    gstate = {}

    def emit_group_setup(g):
        ob = g * OG
        wq_g = wgp.tile([128, KT, OG], BF16, tag="w", name=f"wq_{g}{R}")
        nc.gpsimd.dma_start(wq_g[:], wqt_v[:, :, ob:ob + OG])
        wk_g = wgp.tile([128, KT, OG], BF16, tag="w", name=f"wk_{g}{R}")
        nc.gpsimd.dma_start(wk_g[:], wkt_v[:, :, ob:ob + OG])
        wv_g = wgp.tile([128, KT, OG], BF16, tag="w", name=f"wv_{g}{R}")
        nc.gpsimd.dma_start(wv_g[:], wvt_v[:, :, ob:ob + OG])
        # q projection: qT_g[o_local, m]
        qT_g = qtgp.tile([128, 2, M_SH], BF16, tag="qtg",
                         name=f"qT_{g}{R}")
        kT_g = ktgp.tile([128, 2, N], BF16, tag="ktg", name=f"kT_{g}{R}")
        v_big = vbigp.tile([128, N // 128, HPG, DH + 1], BF16, tag="v",
                           name=f"v_{g}{R}")
        nc.gpsimd.dma_start(
            v_big[:, :, :, DH],
            onesb.rearrange("(a b) -> a b", a=N // 128)[None]
            .to_broadcast((128, N // 128, HPG)))
        if level >= 1:
            for ot in range(2):
                ps = psp.tile([128, M_SH], F32, tag="mm", name=f"psq{g}{ot}{R}")
                for kt in range(KT):
                    nc.tensor.matmul(
                        ps[:], wq_g[:, kt, ot * 128:(ot + 1) * 128],
                        qt_sb[:, kt], start=(kt == 0), stop=(kt == KT - 1))
                with nc.allow_low_precision(reason="bf16 q activations"):
                    nc.vector.tensor_scalar_add(
                        qT_g[:, ot], ps[:], bq_sb[:, 2 * g + ot, None])
        gstate[g] = (wk_g, wv_g, qT_g, kT_g, v_big)

    def emit_kv_chunk(g, ch):
        wk_g, wv_g, qT_g, kT_g, v_big = gstate[g]
        ob = g * OG
        if level < 1:
            return
        for ot in range(2):
            ps = psp.tile([128, 512], F32, tag="mm", name=f"psk{g}{ch}{ot}{R}")
            for kt in range(KT):
                nc.tensor.matmul(
                    ps[:], wk_g[:, kt, ot * 128:(ot + 1) * 128],
                    ck_all[:, kt, ch * 512:(ch + 1) * 512],
                    start=(kt == 0), stop=(kt == KT - 1))
            with nc.allow_low_precision(reason="bf16 k activations"):
                nc.vector.tensor_scalar_add(
                    kT_g[:, ot, ch * 512:(ch + 1) * 512], ps[:],
                    bk_sb[:, 2 * g + ot, None])
        for ntl in range(4):
            nt = ch * 4 + ntl
            psv = psp.tile([128, 512], F32, tag="mm", name=f"psv{g}{nt}{R}")
            for kt in range(KT):
                nc.tensor.matmul(
                    psv[:, 0:OG],
                    ck_all[:, kt, nt * 128:(nt + 1) * 128],
                    wv_g[:, kt], start=(kt == 0), stop=(kt == KT - 1))
            with nc.allow_low_precision(reason="bf16 v activations"):
                nc.vector.tensor_add(
                    out=v_big[:, nt, :, 0:DH],
                    in0=psv[:, 0:OG].rearrange("p (j d) -> p j d", j=HPG),
                    in1=bv_b[:, ob:ob + OG].rearrange(
                        "p (j d) -> p j d", j=HPG))

    def emit_attn_nt(g, wave, nt, pv_ps):
        # one head PAIR per wave iteration: both S^T matmuls write the
        # same two-bank psum so a single wide ACT does exp for both.
        _, _, qT_g, kT_g, v_big = gstate[g]
        j0, j1 = 2 * wave, 2 * wave + 1
        ps2 = s2p.tile([128, 2, M_SH], F32, tag="s2",
                       name=f"pss{g}{wave}{nt}{R}")
        for i, j in enumerate((j0, j1)):
            base, tl = (j % 2) * 64, j // 2
            nc.tensor.matmul(
                ps2[:, i],
                kT_g[base:base + 64, tl, nt * 128:(nt + 1) * 128],
                qT_g[base:base + 64, tl, :],
                start=True, stop=True)
        pt_t = ptp.tile([128, 2, M_SH], BF16, tag="pt",
                        name=f"pt{g}{wave}{nt}{R}")
        with nc.allow_low_precision(reason="bf16 attn probabilities"):
            nc.scalar.activation(
                out=pt_t[:], in_=ps2[:], func=AFT.Exp,
                bias=gate_sb[:, nt, None], scale=SCALE)
        for i, j in enumerate((j0, j1)):
            nc.tensor.matmul(
                pv_ps[i][:], v_big[:, nt, j, :], pt_t[:, i],
                start=(nt == 0), stop=(nt == N // 128 - 1))

    def emit_tails(g, wave, pv_ps):
        # normalise by accumulated denominator row; pack into aot.
        # Drain the pv psum FIRST (recip of the denominator row + raw
        # copy) so its bank frees quickly for the next wave; the
        # broadcast+multiply then run off SBUF/another bank.
        for i, j in enumerate((2 * wave, 2 * wave + 1)):
            recip = small.tile([1, M_SH], F32R, tag="recip",
                               name=f"rc{g}{j}{R}")
            with nc.allow_low_precision(
                    reason="fp32r operand for PE broadcast matmul"):
                nc.vector.reciprocal(recip[:], pv_ps[i][DH:DH + 1, :])
            ao_raw = small.tile([DH, M_SH], F32, tag="ao_raw",
                                name=f"ar{g}{j}{R}")
            nc.scalar.activation(out=ao_raw[:], in_=pv_ps[i][0:DH, :],
                                 func=AFT.Copy, bias=0.0, scale=1.0)
            ps_b = psp.tile([128, M_SH], F32, tag="mm",
                            name=f"psb{g}{j}{R}")
            nc.tensor.matmul(ps_b[0:DH, :], ones1[:], recip[:],
                             start=True, stop=True)
            ao_t = small.tile([DH, M_SH], BF16, tag="aot_tmp",
                              name=f"ao{g}{j}{R}")
            with nc.allow_low_precision(reason="bf16 attn outputs"):
                nc.vector.tensor_mul(out=ao_t[:], in0=ps_b[0:DH, :],
                                     in1=ao_raw[:])
            pb = (j % 2) * 64
            nc.sync.dma_start(
                aot[pb:pb + DH, 2 * g + j // 2, :], ao_t[:])

    # software pipeline: attention of group g interleaves with the
    # k/v projection chunks (and setup) of group g+1, so the PE has
    # independent matmul work whenever it would stall on ACT exp.
    emit_group_setup(0)
    for ch in range(N // 512):
        emit_kv_chunk(0, ch)
    wo_c = []
    for g in range(G):
        if g + 1 < G:
            emit_group_setup(g + 1)
        else:
            # prefetch the o-proj weights into the (now idle) stream
            # slots while the last group's attention runs
            for oc in range(2):
                w = stream.tile([128, KT, 512], BF16, tag="wo",
                                name=f"wo_{oc}{R}")
                nc.gpsimd.dma_start(
                    w[:], wot_v[:, :, oc * 512:(oc + 1) * 512])
                wo_c.append(w)
        for wave in range(2):
            pv_ps = [pvp.tile([DH + 1, M_SH], F32, tag="pv",
                              name=f"pv_{g}_{wave}_{i}{R}")
                     for i in range(2)]
            for nt in range(N // 128):
                if level >= 2:
                    emit_attn_nt(g, wave, nt, pv_ps)
                if g + 1 < G and nt % 8 == 7:
                    emit_kv_chunk(g + 1, 2 * wave + nt // 8)
            if level >= 2:
                emit_tails(g, wave, pv_ps)
        del gstate[g]

    # ---- output projection + bias + residual + LayerNorm ----
    for mt in range(M_SH // 128 if level >= 3 else 0):
        x_t = outst.tile([128, D], F32, tag="x")
        qn_t = outst.tile([128, D], F32, tag="qn")
        nc.sync.dma_start(qn_t[:], qn[mt * 128:(mt + 1) * 128, :])
        for oc in range(2):
            ps = psp.tile([128, 512], F32, tag="mm")
            for kt in range(KT):
                nc.tensor.matmul(
                    ps[:], aot[:, kt, mt * 128:(mt + 1) * 128],
                    wo_c[oc][:, kt], start=(kt == 0), stop=(kt == KT - 1))
            nc.vector.tensor_add(out=x_t[:, oc * 512:(oc + 1) * 512],
                                 in0=ps[:],
                                 in1=bo_b[:, oc * 512:(oc + 1) * 512])
        nc.vector.tensor_add(out=x_t[:], in0=x_t[:], in1=qn_t[:])
        # LayerNorm over D=1024 (two bn_stats subgroups of 512)
        st = outst.tile([128, 2, 6], F32, tag="st")
        nc.vector.bn_stats(st[:, 0], x_t[:, 0:512])
        nc.vector.bn_stats(st[:, 1], x_t[:, 512:1024])
        mv = outst.tile([128, 2], F32, tag="mv")
        nc.vector.bn_aggr(mv[:], st[:])
        nm = outst.tile([128, 1], F32, tag="nm")
        nc.vector.tensor_scalar_mul(nm[:], mv[:, 0:1], -1.0)
        rstd = outst.tile([128, 1], F32, tag="rstd")
        nc.scalar.activation(out=rstd[:], in_=mv[:, 1:2],
                             func=AFT.Sqrt, bias=eps_sb[:], scale=1.0)
        nc.vector.reciprocal(rstd[:], rstd[:])
        nc.vector.tensor_scalar_add(x_t[:], x_t[:], nm[:])
        nc.vector.tensor_scalar_mul(x_t[:], x_t[:], rstd[:])
        nc.vector.tensor_mul(out=x_t[:], in0=x_t[:], in1=gamma_b[:])
        nc.vector.tensor_add(out=x_t[:], in0=x_t[:], in1=beta_b[:])
        nc.sync.dma_start(out[mt * 128:(mt + 1) * 128, :], x_t[:])


def make_in_maps(inputs):
    f = lambda x: np.ascontiguousarray(np.asarray(x, dtype=np.float32))
    bfdt = mybir.dt.np(BF16)
    fb = lambda x: np.ascontiguousarray(
        np.asarray(x, dtype=np.float32).astype(bfdt))
    Q, KV = f(inputs["Q"]), f(inputs["KV"])
    gate = f(inputs["log_gate_bias"])
    shared = {
        "wqt": fb(np.asarray(inputs["Wq"]).T),
        "wkt": fb(np.asarray(inputs["Wk"]).T),
        "wvt": fb(np.asarray(inputs["Wv"]).T),
        "wot": fb(np.asarray(inputs["Wo"]).T),
        "bq": f(inputs["bq"]), "bk": f(inputs["bk"]),
        "bv": f(inputs["bv"]), "bo": f(inputs["bo"]),
        "gamma": f(inputs["gamma"]), "beta": f(inputs["beta"]),
        "onesc": np.ones(DH, dtype=np.float32),
        "onesb": np.ones(DH, dtype=np.float32).astype(bfdt),
    }
    in_maps = []
    for c in range(N_CORES):
        b, mh = c // 2, c % 2
        in_maps.append({
            "qt": fb(Q[b].T[:, mh * M_SH:(mh + 1) * M_SH]),
            "qn": f(Q[b, mh * M_SH:(mh + 1) * M_SH, :]),
            "kvt": fb(KV[b].T),
            "gate": np.ascontiguousarray(gate[b]),
            **shared,
        })
    return in_maps


def assemble(results):
    out = np.empty((B, M, D), dtype=np.float32)
    for c in range(N_CORES):
        b, mh = c // 2, c % 2
        out[b, mh * M_SH:(mh + 1) * M_SH, :] = results[c]["out"]
    return out


def kernel(**inputs) -> np.ndarray:
    if "nc" not in _CACHE:
        _CACHE["nc"] = build_nc()
    nc = _CACHE["nc"]
    in_maps = make_in_maps(inputs)
    res = run_bass_kernel_spmd(nc, in_maps, core_ids=list(range(N_CORES)))
    return assemble(res.results)


# revision 9
# speedup vs baseline: 9.7501x; 1.4582x over previous
"""Cross-attention block (q/k/v proj -> gated softmax attention -> out proj
-> residual + LayerNorm) on 8 Trainium2 NeuronCores.

Sharding: data-parallel over batch (B=4) x query-row halves (2) = 8 shards.
Each core handles one (b, m-half): computes full k/v projections for its
batch, attention for all 16 heads over its 512 query rows, output
projection, residual + LayerNorm. No collectives; host concatenates.

Layout strategy:
  - All matmul operands are bf16 (cast on host); every accumulation stays
    fp32 in PSUM and softmax/LayerNorm math is fp32. Error ~1e-3
    scale-relative (gate is 2e-2) while halving DMA traffic.
  - Inputs are PRE-PACKED on host into the exact SBUF destination layout
    [128 partitions, ...], so every input tile loads with one fully
    contiguous dma_start (measured: the un-packed strided loads cost
    ~360us/exec; DMA here is latency/descriptor-bound, not byte-bound).
  - KV^T is loaded into SBUF ONCE per execution (bf16, 4MB); all four
    head-group k/v projections stream it from SBUF.
  - q/k projections produce head-major (o-major) tiles directly:
      qT [o, m], kT [o, n]  -> exactly the lhsT/rhs layouts attention needs.
  - Scores are computed TRANSPOSED: S^T[n, m] = kT_slice.T @ qT_slice, so
    softmax's additive gate bias (per n) is a per-partition ACT bias and
    exp(S/8 + gate) is a single fused ACT op from PSUM. No row-max pass
    (scores are O(+-8); exp is fp32-safe).
  - v is produced in natural [n, dh] layout with a ones-column appended per
    head, generated BY the v-projection itself: Wv is augmented with a zero
    column per head and bv with a 1.0 there, so attn@v AND the softmax
    denominator accumulate in one PV matmul: psum[65, m] per head.
  - PV output [dh, m] is k-major: stacked heads form the o-proj lhsT with
    no transposes anywhere in the kernel.
"""
import numpy as np

import concourse.bass as bass
import concourse.mybir as mybir
import concourse.tile as tile
from concourse import bacc
from concourse.bass_utils import run_bass_kernel_spmd

F32 = mybir.dt.float32
F32R = mybir.dt.float32r
BF16 = mybir.dt.bfloat16
AFT = mybir.ActivationFunctionType

B, M, N, D = 4, 1024, 2048, 1024
H, DH = 16, 64
M_SH = M // 2          # query rows per core
G = 4                  # head groups
HPG = H // G           # heads per group
OG = HPG * DH          # 256 output cols per group
OGA = HPG * (DH + 1)   # 260: v-proj cols incl. per-head ones column
KT = D // 128          # 8 contraction subtiles
WSEG = 2 * KT * OG + KT * OGA   # 6176 cols per group in the packed weights
LN_EPS = 1e-5
SCALE = 1.0 / np.sqrt(DH)
N_CORES = 8

_CACHE = {}


def build_nc(reps=1, level=3):
    """Build the per-core Bass program.

    reps>1 emits the full kernel body `reps` times into one NEFF (same
    output, recomputed). Used only for timing: the slope between a
    reps=R and reps=1 dispatch isolates true on-device execution time
    from the constant host->device dispatch overhead of this setup.

    level (timing experiments only; <3 gives wrong results): 0 = input
    DMA loads only, 1 = +q/k/v projections, 2 = +attention, 3 = full.
    """
    nc = bacc.Bacc("TRN2", target_bir_lowering=False, debug=False)

    # pre-packed inputs (see make_in_maps): partition-major, contiguous
    qtp = nc.dram_tensor("qtp", [128, KT, M_SH], BF16, kind="ExternalInput")
    ckp_d = nc.dram_tensor("ckp", [128, KT, N], BF16, kind="ExternalInput")
    wall = nc.dram_tensor("wall", [128, G, WSEG], BF16, kind="ExternalInput")
    wop = nc.dram_tensor("wop", [128, 2, KT, 512], BF16, kind="ExternalInput")
    qn = nc.dram_tensor("qn", [M_SH, D], F32, kind="ExternalInput")
    gate = nc.dram_tensor("gate", [N], F32, kind="ExternalInput")
    bq = nc.dram_tensor("bq", [D], F32, kind="ExternalInput")
    bk = nc.dram_tensor("bk", [D], F32, kind="ExternalInput")
    bva = nc.dram_tensor("bva", [G * OGA], F32, kind="ExternalInput")
    bo = nc.dram_tensor("bo", [D], F32, kind="ExternalInput")
    gamma = nc.dram_tensor("gamma", [D], F32, kind="ExternalInput")
    beta = nc.dram_tensor("beta", [D], F32, kind="ExternalInput")
    onesc = nc.dram_tensor("onesc", [DH], F32, kind="ExternalInput")
    out = nc.dram_tensor("out", [M_SH, D], F32, kind="ExternalOutput")

    with tile.TileContext(nc) as tc:
        with tc.tile_pool(name="consts", bufs=1) as consts, \
             tc.tile_pool(name="ckall", bufs=1) as ckp, \
             tc.tile_pool(name="stream", bufs=2) as stream, \
             tc.tile_pool(name="wg", bufs=2) as wgp, \
             tc.tile_pool(name="qtg", bufs=2) as qtgp, \
             tc.tile_pool(name="ktg", bufs=2) as ktgp, \
             tc.tile_pool(name="vbig", bufs=2) as vbigp, \
             tc.tile_pool(name="pt", bufs=2) as ptp, \
             tc.tile_pool(name="aot", bufs=1) as aotp, \
             tc.tile_pool(name="small", bufs=2) as small, \
             tc.tile_pool(name="outst", bufs=1) as outst, \
             tc.tile_pool(name="ps", bufs=2, space="PSUM") as psp, \
             tc.tile_pool(name="s2", bufs=2, space="PSUM") as s2p, \
             tc.tile_pool(name="pv", bufs=2, space="PSUM") as pvp:

            # ---- constants ----
            gate_sb = consts.tile([128, N // 128], F32, tag="gate")
            nc.sync.dma_start(gate_sb[:], gate.rearrange("(t p) -> p t", p=128))
            bq_sb = consts.tile([128, KT], F32, tag="bq")
            nc.sync.dma_start(bq_sb[:], bq.rearrange("(t p) -> p t", p=128))
            bk_sb = consts.tile([128, KT], F32, tag="bk")
            nc.sync.dma_start(bk_sb[:], bk.rearrange("(t p) -> p t", p=128))
            bva_b = consts.tile([128, G * OGA], F32, tag="bva")
            nc.sync.dma_start(bva_b[:], bva[None, :].to_broadcast((128, G * OGA)))
            bo_b = consts.tile([128, D], F32, tag="bo")
            nc.sync.dma_start(bo_b[:], bo[None, :].to_broadcast((128, D)))
            gamma_b = consts.tile([128, D], F32, tag="gamma")
            nc.sync.dma_start(gamma_b[:], gamma[None, :].to_broadcast((128, D)))
            beta_b = consts.tile([128, D], F32, tag="beta")
            nc.sync.dma_start(beta_b[:], beta[None, :].to_broadcast((128, D)))
            eps_sb = consts.tile([128, 1], F32, tag="eps")
            nc.vector.memset(eps_sb[:], LN_EPS)
            ones1 = consts.tile([1, DH], F32R, tag="ones1")
            nc.gpsimd.dma_start(ones1[:], onesc[None, :])

            for rep in range(reps):
                emit_rep(nc, rep, level, consts, ckp, stream, wgp, qtgp, ktgp,
                         vbigp, ptp, aotp, small, outst, psp, s2p, pvp,
                         qtp, ckp_d, wall, wop, qn, out,
                         gate_sb, bq_sb, bk_sb, bva_b, bo_b,
                         gamma_b, beta_b, eps_sb, ones1)

    nc.compile()
    return nc


def emit_rep(nc, rep, level, consts, ckp, stream, wgp, qtgp, ktgp, vbigp,
             ptp, aotp, small, outst, psp, s2p, pvp,
             qtp, ckp_d, wall, wop, qn, out,
             gate_sb, bq_sb, bk_sb, bva_b, bo_b, gamma_b, beta_b,
             eps_sb, ones1):
    """One full kernel body (q/k/v proj -> attention -> o-proj -> LN)."""
    R = f"r{rep}"

    # q activations (1 contiguous load)
    qt_sb = consts.tile([128, KT, M_SH], BF16, tag="qt")
    nc.gpsimd.dma_start(qt_sb[:], qtp[:])

    # KV^T resident in SBUF, loaded once (bf16, 4MB, contiguous)
    ck_all = ckp.tile([128, KT, N], BF16, tag="ckall")
    nc.gpsimd.dma_start(ck_all[:], ckp_d[:])

    # o-proj accumulator input: stacked head outputs [o, m]
    aot = aotp.tile([128, KT, M_SH], BF16, tag="aot")

    gstate = {}

    def emit_group_setup(g):
        # one contiguous load for this group's wq|wk|wv(augmented)
        wg_t = wgp.tile([128, WSEG], BF16, tag="w", name=f"w_{g}{R}")
        nc.gpsimd.dma_start(wg_t[:], wall[:, g])
        wq = lambda kt, c0, c1: wg_t[:, kt * OG + c0:kt * OG + c1]
        wk = lambda kt, c0, c1: wg_t[:, KT * OG + kt * OG + c0:
                                     KT * OG + kt * OG + c1]
        wv = lambda kt: wg_t[:, 2 * KT * OG + kt * OGA:
                             2 * KT * OG + (kt + 1) * OGA]
        # q projection: qT_g[o_local, m]
        qT_g = qtgp.tile([128, 2, M_SH], BF16, tag="qtg",
                         name=f"qT_{g}{R}")
        kT_g = ktgp.tile([128, 2, N], BF16, tag="ktg", name=f"kT_{g}{R}")
        v_big = vbigp.tile([128, N // 128, HPG, DH + 1], BF16, tag="v",
                           name=f"v_{g}{R}")
        if level >= 1:
            for ot in range(2):
                ps = psp.tile([128, M_SH], F32, tag="mm", name=f"psq{g}{ot}{R}")
                for kt in range(KT):
                    nc.tensor.matmul(
                        ps[:], wq(kt, ot * 128, (ot + 1) * 128),
                        qt_sb[:, kt], start=(kt == 0), stop=(kt == KT - 1))
                with nc.allow_low_precision(reason="bf16 q activations"):
                    nc.vector.tensor_scalar_add(
                        qT_g[:, ot], ps[:], bq_sb[:, 2 * g + ot, None])
        gstate[g] = (wk, wv, qT_g, kT_g, v_big)

    def emit_kv_chunk(g, ch):
        wk, wv, qT_g, kT_g, v_big = gstate[g]
        ob = g * OGA
        if level < 1:
            return
        for ot in range(2):
            ps = psp.tile([128, 512], F32, tag="mm", name=f"psk{g}{ch}{ot}{R}")
            for kt in range(KT):
                nc.tensor.matmul(
                    ps[:], wk(kt, ot * 128, (ot + 1) * 128),
                    ck_all[:, kt, ch * 512:(ch + 1) * 512],
                    start=(kt == 0), stop=(kt == KT - 1))
            with nc.allow_low_precision(reason="bf16 k activations"):
                nc.vector.tensor_scalar_add(
                    kT_g[:, ot, ch * 512:(ch + 1) * 512], ps[:],
                    bk_sb[:, 2 * g + ot, None])
        for ntl in range(4):
            nt = ch * 4 + ntl
            psv = psp.tile([128, 512], F32, tag="mm", name=f"psv{g}{nt}{R}")
            for kt in range(KT):
                nc.tensor.matmul(
                    psv[:, 0:OGA],
                    ck_all[:, kt, nt * 128:(nt + 1) * 128],
                    wv(kt), start=(kt == 0), stop=(kt == KT - 1))
            with nc.allow_low_precision(reason="bf16 v activations"):
                nc.vector.tensor_add(
                    out=v_big[:, nt],
                    in0=psv[:, 0:OGA].rearrange("p (j d) -> p j d", j=HPG),
                    in1=bva_b[:, ob:ob + OGA].rearrange(
                        "p (j d) -> p j d", j=HPG))

    def emit_attn_nt(g, wave, nt, pv_ps):
        # one head PAIR per wave iteration: both S^T matmuls write the
        # same two-bank psum so a single wide ACT does exp for both.
        _, _, qT_g, kT_g, v_big = gstate[g]
        j0, j1 = 2 * wave, 2 * wave + 1
        ps2 = s2p.tile([128, 2, M_SH], F32, tag="s2",
                       name=f"pss{g}{wave}{nt}{R}")
        for i, j in enumerate((j0, j1)):
            base, tl = (j % 2) * 64, j // 2
            nc.tensor.matmul(
                ps2[:, i],
                kT_g[base:base + 64, tl, nt * 128:(nt + 1) * 128],
                qT_g[base:base + 64, tl, :],
                start=True, stop=True)
        pt_t = ptp.tile([128, 2, M_SH], BF16, tag="pt",
                        name=f"pt{g}{wave}{nt}{R}")
        with nc.allow_low_precision(reason="bf16 attn probabilities"):
            nc.scalar.activation(
                out=pt_t[:], in_=ps2[:], func=AFT.Exp,
                bias=gate_sb[:, nt, None], scale=SCALE)
        for i, j in enumerate((j0, j1)):
            nc.tensor.matmul(
                pv_ps[i][:], v_big[:, nt, j, :], pt_t[:, i],
                start=(nt == 0), stop=(nt == N // 128 - 1))

    def emit_tails(g, wave, pv_ps):
        # normalise by accumulated denominator row; pack into aot.
        # Drain the pv psum FIRST (recip of the denominator row + raw
        # copy) so its bank frees quickly for the next wave; the
        # broadcast+multiply then run off SBUF/another bank.
        for i, j in enumerate((2 * wave, 2 * wave + 1)):
            recip = small.tile([1, M_SH], F32R, tag="recip",
                               name=f"rc{g}{j}{R}")
            with nc.allow_low_precision(
                    reason="fp32r operand for PE broadcast matmul"):
                nc.vector.reciprocal(recip[:], pv_ps[i][DH:DH + 1, :])
            ao_raw = small.tile([DH, M_SH], F32, tag="ao_raw",
                                name=f"ar{g}{j}{R}")
            nc.scalar.activation(out=ao_raw[:], in_=pv_ps[i][0:DH, :],
                                 func=AFT.Copy, bias=0.0, scale=1.0)
            ps_b = psp.tile([128, M_SH], F32, tag="mm",
                            name=f"psb{g}{j}{R}")
            nc.tensor.matmul(ps_b[0:DH, :], ones1[:], recip[:],
                             start=True, stop=True)
            ao_t = small.tile([DH, M_SH], BF16, tag="aot_tmp",
                              name=f"ao{g}{j}{R}")
            with nc.allow_low_precision(reason="bf16 attn outputs"):
                nc.vector.tensor_mul(out=ao_t[:], in0=ps_b[0:DH, :],
                                     in1=ao_raw[:])
            pb = (j % 2) * 64
            nc.sync.dma_start(
                aot[pb:pb + DH, 2 * g + j // 2, :], ao_t[:])

    # software pipeline: attention of group g interleaves with the
    # k/v projection chunks (and setup) of group g+1, so the PE has
    # independent matmul work whenever it would stall on ACT exp.
    emit_group_setup(0)
    for ch in range(N // 512):
        emit_kv_chunk(0, ch)
    wo_c = []
    for g in range(G):
        if g + 1 < G:
            emit_group_setup(g + 1)
        else:
            # prefetch the o-proj weights into the (now idle) stream
            # slots while the last group's attention runs
            for oc in range(2):
                w = stream.tile([128, KT, 512], BF16, tag="wo",
                                name=f"wo_{oc}{R}")
                nc.gpsimd.dma_start(w[:], wop[:, oc])
                wo_c.append(w)
        for wave in range(2):
            pv_ps = [pvp.tile([DH + 1, M_SH], F32, tag="pv",
                              name=f"pv_{g}_{wave}_{i}{R}")
                     for i in range(2)]
            for nt in range(N // 128):
                if level >= 2:
                    emit_attn_nt(g, wave, nt, pv_ps)
                if g + 1 < G and nt % 8 == 7:
                    emit_kv_chunk(g + 1, 2 * wave + nt // 8)
            if level >= 2:
                emit_tails(g, wave, pv_ps)
        del gstate[g]

    # ---- output projection + bias + residual + LayerNorm ----
    for mt in range(M_SH // 128 if level >= 3 else 0):
        x_t = outst.tile([128, D], F32, tag="x")
        qn_t = outst.tile([128, D], F32, tag="qn")
        nc.scalar.dma_start(qn_t[:], qn[mt * 128:(mt + 1) * 128, :])
        for oc in range(2):
            ps = psp.tile([128, 512], F32, tag="mm")
            for kt in range(KT):
                nc.tensor.matmul(
                    ps[:], aot[:, kt, mt * 128:(mt + 1) * 128],
                    wo_c[oc][:, kt], start=(kt == 0), stop=(kt == KT - 1))
            nc.vector.tensor_add(out=x_t[:, oc * 512:(oc + 1) * 512],
                                 in0=ps[:],
                                 in1=bo_b[:, oc * 512:(oc + 1) * 512])
        nc.vector.tensor_add(out=x_t[:], in0=x_t[:], in1=qn_t[:])
        # LayerNorm over D=1024 (two bn_stats subgroups of 512)
        st = outst.tile([128, 2, 6], F32, tag="st")
        nc.vector.bn_stats(st[:, 0], x_t[:, 0:512])
        nc.vector.bn_stats(st[:, 1], x_t[:, 512:1024])
        mv = outst.tile([128, 2], F32, tag="mv")
        nc.vector.bn_aggr(mv[:], st[:])
        nm = outst.tile([128, 1], F32, tag="nm")
        nc.vector.tensor_scalar_mul(nm[:], mv[:, 0:1], -1.0)
        rstd = outst.tile([128, 1], F32, tag="rstd")
        nc.scalar.activation(out=rstd[:], in_=mv[:, 1:2],
                             func=AFT.Sqrt, bias=eps_sb[:], scale=1.0)
        nc.vector.reciprocal(rstd[:], rstd[:])
        nc.vector.tensor_scalar_add(x_t[:], x_t[:], nm[:])
        nc.vector.tensor_scalar_mul(x_t[:], x_t[:], rstd[:])
        nc.vector.tensor_mul(out=x_t[:], in0=x_t[:], in1=gamma_b[:])
        nc.vector.tensor_add(out=x_t[:], in0=x_t[:], in1=beta_b[:])
        nc.sync.dma_start(out[mt * 128:(mt + 1) * 128, :], x_t[:])


def _pack128(a_T, bfdt):
    """[D_in, X] contraction-major -> [128, D_in//128, X] partition-major."""
    ktl = a_T.shape[0] // 128
    return np.ascontiguousarray(
        a_T.reshape(ktl, 128, -1).transpose(1, 0, 2).astype(bfdt))


def make_in_maps(inputs):
    f = lambda x: np.ascontiguousarray(np.asarray(x, dtype=np.float32))
    bfdt = mybir.dt.np(BF16)
    Q, KV = f(inputs["Q"]), f(inputs["KV"])
    gate = f(inputs["log_gate_bias"])
    wq_T = f(np.asarray(inputs["Wq"]).T)   # [D_in, D_out]
    wk_T = f(np.asarray(inputs["Wk"]).T)
    wv_T = f(np.asarray(inputs["Wv"]).T)
    wo_T = f(np.asarray(inputs["Wo"]).T)

    # packed per-group weights: wq | wk | wv_augmented (ones col per head)
    wall = np.empty((128, G, WSEG), dtype=bfdt)
    for g in range(G):
        ob = g * OG
        wq_g = _pack128(wq_T[:, ob:ob + OG], bfdt).reshape(128, KT * OG)
        wk_g = _pack128(wk_T[:, ob:ob + OG], bfdt).reshape(128, KT * OG)
        wv_g = wv_T[:, ob:ob + OG].reshape(D, HPG, DH)
        wv_aug = np.zeros((D, HPG, DH + 1), np.float32)
        wv_aug[:, :, 0:DH] = wv_g
        wv_gp = _pack128(wv_aug.reshape(D, OGA), bfdt).reshape(128, KT * OGA)
        wall[:, g, 0:KT * OG] = wq_g
        wall[:, g, KT * OG:2 * KT * OG] = wk_g
        wall[:, g, 2 * KT * OG:] = wv_gp
    wop = _pack128(wo_T, bfdt).reshape(128, KT, 2, 512).transpose(
        0, 2, 1, 3)   # [128, oc, kt, 512]
    wop = np.ascontiguousarray(wop)

    bva = np.zeros((G, HPG, DH + 1), np.float32)
    bva[:, :, 0:DH] = f(inputs["bv"]).reshape(G, HPG, DH)
    bva[:, :, DH] = 1.0

    shared = {
        "wall": wall, "wop": wop,
        "bq": f(inputs["bq"]), "bk": f(inputs["bk"]),
        "bva": np.ascontiguousarray(bva.reshape(-1)),
        "bo": f(inputs["bo"]),
        "gamma": f(inputs["gamma"]), "beta": f(inputs["beta"]),
        "onesc": np.ones(DH, dtype=np.float32),
    }
    in_maps = []
    ck_cache = {}
    for c in range(N_CORES):
        b, mh = c // 2, c % 2
        if b not in ck_cache:
            ck_cache[b] = _pack128(KV[b].T, bfdt)
        in_maps.append({
            "qtp": _pack128(Q[b].T[:, mh * M_SH:(mh + 1) * M_SH], bfdt),
            "qn": f(Q[b, mh * M_SH:(mh + 1) * M_SH, :]),
            "ckp": ck_cache[b],
            "gate": np.ascontiguousarray(gate[b]),
            **shared,
        })
    return in_maps


def assemble(results):
    out = np.empty((B, M, D), dtype=np.float32)
    for c in range(N_CORES):
        b, mh = c // 2, c % 2
        out[b, mh * M_SH:(mh + 1) * M_SH, :] = results[c]["out"]
    return out


def kernel(**inputs) -> np.ndarray:
    if "nc" not in _CACHE:
        _CACHE["nc"] = build_nc()
    nc = _CACHE["nc"]
    in_maps = make_in_maps(inputs)
    res = run_bass_kernel_spmd(nc, in_maps, core_ids=list(range(N_CORES)))
    return assemble(res.results)


# revision 11
# speedup vs baseline: 26.2116x; 2.6883x over previous
"""Cross-attention block (q/k/v proj -> gated softmax attention -> out proj
-> residual + LayerNorm) on 8 Trainium2 NeuronCores.

Sharding: data-parallel over batch (B=4) x query-row halves (2) = 8 shards.
Each core handles one (b, m-half): computes full k/v projections for its
batch, attention for all 16 heads over its 512 query rows, output
projection, residual + LayerNorm. No collectives; host concatenates.

Layout strategy:
  - All matmul operands are bf16 (cast on host); every accumulation stays
    fp32 in PSUM and softmax/LayerNorm math is fp32. Error ~1e-3
    scale-relative (gate is 2e-2) while halving DMA traffic.
  - Inputs are PRE-PACKED on host into the exact SBUF destination layout
    [128 partitions, ...], so every input tile loads with one fully
    contiguous dma_start (measured: the un-packed strided loads cost
    ~360us/exec; DMA here is latency/descriptor-bound, not byte-bound).
  - KV^T is loaded into SBUF ONCE per execution (bf16, 4MB); all four
    head-group k/v projections stream it from SBUF.
  - q/k projections produce head-major (o-major) tiles directly:
      qT [o, m], kT [o, n]  -> exactly the lhsT/rhs layouts attention needs.
  - Scores are computed TRANSPOSED: S^T[n, m] = kT_slice.T @ qT_slice, so
    softmax's additive gate bias (per n) is a per-partition ACT bias and
    exp(S/8 + gate) is a single fused ACT op from PSUM. No row-max pass
    (scores are O(+-8); exp is fp32-safe).
  - v is produced in natural [n, dh] layout with a ones-column appended per
    head, generated BY the v-projection itself: Wv is augmented with a zero
    column per head and bv with a 1.0 there, so attn@v AND the softmax
    denominator accumulate in one PV matmul: psum[65, m] per head.
  - PV output [dh, m] is k-major: stacked heads form the o-proj lhsT with
    no transposes anywhere in the kernel.
"""
import numpy as np

import concourse.bass as bass
import concourse.mybir as mybir
import concourse.tile as tile
from concourse import bacc
from concourse.bass_utils import run_bass_kernel_spmd

F32 = mybir.dt.float32
F32R = mybir.dt.float32r
BF16 = mybir.dt.bfloat16
AFT = mybir.ActivationFunctionType

B, M, N, D = 4, 1024, 2048, 1024
H, DH = 16, 64
M_SH = M // 2          # query rows per core
G = 4                  # head groups
HPG = H // G           # heads per group
OG = HPG * DH          # 256 output cols per group
OGA = HPG * (DH + 1)   # 260: v-proj cols incl. per-head ones column
KT = D // 128          # 8 contraction subtiles
WSEG = 2 * KT * OG + KT * OGA   # 6176 cols per group in the packed weights
LN_EPS = 1e-5
SCALE = 1.0 / np.sqrt(DH)
N_CORES = 8

_CACHE = {}


def build_nc(reps=1, level=3):
    """Build the per-core Bass program.

    reps>1 emits the full kernel body `reps` times into one NEFF (same
    output, recomputed). Used only for timing: the slope between a
    reps=R and reps=1 dispatch isolates true on-device execution time
    from the constant host->device dispatch overhead of this setup.

    level (timing experiments only; <3 gives wrong results): 0 = input
    DMA loads only, 1 = +q/k/v projections, 2 = +attention, 3 = full.
    """
    nc = bacc.Bacc("TRN2", target_bir_lowering=False, debug=False)

    # pre-packed inputs (see make_in_maps): partition-major, contiguous
    qtp = nc.dram_tensor("qtp", [128, KT, M_SH], BF16, kind="ExternalInput")
    ckp_d = nc.dram_tensor("ckp", [128, KT, N], BF16, kind="ExternalInput")
    wall = nc.dram_tensor("wall", [128, G, WSEG], BF16, kind="ExternalInput")
    wop = nc.dram_tensor("wop", [128, 2, KT, 512], BF16, kind="ExternalInput")
    qn = nc.dram_tensor("qn", [M_SH, D], F32, kind="ExternalInput")
    gate = nc.dram_tensor("gate", [N], F32, kind="ExternalInput")
    bq = nc.dram_tensor("bq", [D], F32, kind="ExternalInput")
    bk = nc.dram_tensor("bk", [D], F32, kind="ExternalInput")
    bva = nc.dram_tensor("bva", [G * OGA], F32, kind="ExternalInput")
    bo = nc.dram_tensor("bo", [D], F32, kind="ExternalInput")
    gamma = nc.dram_tensor("gamma", [D], F32, kind="ExternalInput")
    beta = nc.dram_tensor("beta", [D], F32, kind="ExternalInput")
    out = nc.dram_tensor("out", [M_SH, D], F32, kind="ExternalOutput")

    with tile.TileContext(nc) as tc:
        with tc.tile_pool(name="consts", bufs=1) as consts, \
             tc.tile_pool(name="ckall", bufs=1) as ckp, \
             tc.tile_pool(name="stream", bufs=2) as stream, \
             tc.tile_pool(name="wg", bufs=2) as wgp, \
             tc.tile_pool(name="qtg", bufs=2) as qtgp, \
             tc.tile_pool(name="ktg", bufs=2) as ktgp, \
             tc.tile_pool(name="vbig", bufs=2) as vbigp, \
             tc.tile_pool(name="pt", bufs=2) as ptp, \
             tc.tile_pool(name="aot", bufs=1) as aotp, \
             tc.tile_pool(name="small", bufs=2) as small, \
             tc.tile_pool(name="outst", bufs=2) as outst, \
             tc.tile_pool(name="ps", bufs=2, space="PSUM") as psp, \
             tc.tile_pool(name="s2", bufs=2, space="PSUM") as s2p, \
             tc.tile_pool(name="pv", bufs=2, space="PSUM") as pvp:

            # ---- constants ----
            gate_sb = consts.tile([128, N // 128], F32, tag="gate")
            nc.sync.dma_start(gate_sb[:], gate.rearrange("(t p) -> p t", p=128))
            bq_sb = consts.tile([128, KT], F32, tag="bq")
            nc.sync.dma_start(bq_sb[:], bq.rearrange("(t p) -> p t", p=128))
            bk_sb = consts.tile([128, KT], F32, tag="bk")
            nc.sync.dma_start(bk_sb[:], bk.rearrange("(t p) -> p t", p=128))
            bva_b = consts.tile([128, G * OGA], F32, tag="bva")
            nc.sync.dma_start(bva_b[:], bva[None, :].to_broadcast((128, G * OGA)))
            bo_b = consts.tile([128, D], F32, tag="bo")
            nc.sync.dma_start(bo_b[:], bo[None, :].to_broadcast((128, D)))
            gamma_b = consts.tile([128, D], F32, tag="gamma")
            nc.sync.dma_start(gamma_b[:], gamma[None, :].to_broadcast((128, D)))
            beta_b = consts.tile([128, D], F32, tag="beta")
            nc.sync.dma_start(beta_b[:], beta[None, :].to_broadcast((128, D)))
            eps_sb = consts.tile([128, 1], F32, tag="eps")
            nc.vector.memset(eps_sb[:], LN_EPS)

            for rep in range(reps):
                emit_rep(nc, rep, level, consts, ckp, stream, wgp, qtgp, ktgp,
                         vbigp, ptp, aotp, small, outst, psp, s2p, pvp,
                         qtp, ckp_d, wall, wop, qn, out,
                         gate_sb, bq_sb, bk_sb, bva_b, bo_b,
                         gamma_b, beta_b, eps_sb)

    nc.compile()
    return nc


def emit_rep(nc, rep, level, consts, ckp, stream, wgp, qtgp, ktgp, vbigp,
             ptp, aotp, small, outst, psp, s2p, pvp,
             qtp, ckp_d, wall, wop, qn, out,
             gate_sb, bq_sb, bk_sb, bva_b, bo_b, gamma_b, beta_b,
             eps_sb):
    """One full kernel body (q/k/v proj -> attention -> o-proj -> LN)."""
    R = f"r{rep}"

    # q activations (1 contiguous load)
    qt_sb = consts.tile([128, KT, M_SH], BF16, tag="qt")
    nc.gpsimd.dma_start(qt_sb[:], qtp[:])

    # KV^T resident in SBUF, loaded once (bf16, 4MB, contiguous)
    ck_all = ckp.tile([128, KT, N], BF16, tag="ckall")
    nc.gpsimd.dma_start(ck_all[:], ckp_d[:])

    # o-proj accumulator input: stacked head outputs [o, m]
    aot = aotp.tile([128, KT, M_SH], BF16, tag="aot")

    gstate = {}

    def emit_group_setup(g):
        # one contiguous load for this group's wq|wk|wv(augmented)
        wg_t = wgp.tile([128, WSEG], BF16, tag="w", name=f"w_{g}{R}")
        nc.gpsimd.dma_start(wg_t[:], wall[:, g])
        wq = lambda kt, c0, c1: wg_t[:, kt * OG + c0:kt * OG + c1]
        wk = lambda kt, c0, c1: wg_t[:, KT * OG + kt * OG + c0:
                                     KT * OG + kt * OG + c1]
        wv = lambda kt: wg_t[:, 2 * KT * OG + kt * OGA:
                             2 * KT * OG + (kt + 1) * OGA]
        # q projection: qT_g[o_local, m]
        qT_g = qtgp.tile([128, 2, M_SH], BF16, tag="qtg",
                         name=f"qT_{g}{R}")
        kT_g = ktgp.tile([128, 2, N], BF16, tag="ktg", name=f"kT_{g}{R}")
        v_big = vbigp.tile([128, N // 128, HPG, DH + 1], BF16, tag="v",
                           name=f"v_{g}{R}")
        if level >= 1:
            for ot in range(2):
                ps = psp.tile([128, M_SH], F32, tag="mm", name=f"psq{g}{ot}{R}")
                for kt in range(KT):
                    nc.tensor.matmul(
                        ps[:], wq(kt, ot * 128, (ot + 1) * 128),
                        qt_sb[:, kt], start=(kt == 0), stop=(kt == KT - 1))
                with nc.allow_low_precision(reason="bf16 q activations"):
                    nc.vector.tensor_scalar_add(
                        qT_g[:, ot], ps[:], bq_sb[:, 2 * g + ot, None])
        gstate[g] = (wk, wv, qT_g, kT_g, v_big)

    def emit_kv_chunk(g, ch):
        wk, wv, qT_g, kT_g, v_big = gstate[g]
        ob = g * OGA
        if level < 1:
            return
        for ot in range(2):
            ps = psp.tile([128, 512], F32, tag="mm", name=f"psk{g}{ch}{ot}{R}")
            for kt in range(KT):
                nc.tensor.matmul(
                    ps[:], wk(kt, ot * 128, (ot + 1) * 128),
                    ck_all[:, kt, ch * 512:(ch + 1) * 512],
                    start=(kt == 0), stop=(kt == KT - 1))
            with nc.allow_low_precision(reason="bf16 k activations"):
                nc.vector.tensor_scalar_add(
                    kT_g[:, ot, ch * 512:(ch + 1) * 512], ps[:],
                    bk_sb[:, 2 * g + ot, None])
        for ntl in range(4):
            nt = ch * 4 + ntl
            psv = psp.tile([128, 512], F32, tag="mm", name=f"psv{g}{nt}{R}")
            for kt in range(KT):
                nc.tensor.matmul(
                    psv[:, 0:OGA],
                    ck_all[:, kt, nt * 128:(nt + 1) * 128],
                    wv(kt), start=(kt == 0), stop=(kt == KT - 1))
            with nc.allow_low_precision(reason="bf16 v activations"):
                nc.vector.tensor_add(
                    out=v_big[:, nt],
                    in0=psv[:, 0:OGA].rearrange("p (j d) -> p j d", j=HPG),
                    in1=bva_b[:, ob:ob + OGA].rearrange(
                        "p (j d) -> p j d", j=HPG))

    def emit_attn_nt(g, wave, nt, pv_ps):
        # one head PAIR per wave iteration: both S^T matmuls write the
        # same two-bank psum so a single wide ACT does exp for both.
        _, _, qT_g, kT_g, v_big = gstate[g]
        j0, j1 = 2 * wave, 2 * wave + 1
        ps2 = s2p.tile([128, 2, M_SH], F32, tag="s2",
                       name=f"pss{g}{wave}{nt}{R}")
        for i, j in enumerate((j0, j1)):
            base, tl = (j % 2) * 64, j // 2
            nc.tensor.matmul(
                ps2[:, i],
                kT_g[base:base + 64, tl, nt * 128:(nt + 1) * 128],
                qT_g[base:base + 64, tl, :],
                start=True, stop=True)
        pt_t = ptp.tile([128, 2, M_SH], BF16, tag="pt",
                        name=f"pt{g}{wave}{nt}{R}")
        with nc.allow_low_precision(reason="bf16 attn probabilities"):
            nc.scalar.activation(
                out=pt_t[:], in_=ps2[:], func=AFT.Exp,
                bias=gate_sb[:, nt, None], scale=SCALE)
        for i, j in enumerate((j0, j1)):
            nc.tensor.matmul(
                pv_ps[i][:], v_big[:, nt, j, :], pt_t[:, i],
                start=(nt == 0), stop=(nt == N // 128 - 1))

    def emit_tails(g, wave, pv_ps):
        # normalise by accumulated denominator row; pack into aot.
        # Drain the pv psum FIRST (recip of the denominator row + raw
        # copy) so its bank frees quickly for the next wave; the
        # broadcast+multiply then run off SBUF/another bank.
        for i, j in enumerate((2 * wave, 2 * wave + 1)):
            recip = small.tile([1, M_SH], F32, tag="recip",
                               name=f"rc{g}{j}{R}")
            nc.vector.reciprocal(recip[:], pv_ps[i][DH:DH + 1, :])
            rb = small.tile([DH, M_SH], F32, tag="rb", name=f"rb{g}{j}{R}")
            nc.gpsimd.partition_broadcast(rb[:], recip[:])
            ao_t = small.tile([DH, M_SH], BF16, tag="aot_tmp",
                              name=f"ao{g}{j}{R}")
            with nc.allow_low_precision(reason="bf16 attn outputs"):
                nc.vector.tensor_mul(out=ao_t[:], in0=rb[:],
                                     in1=pv_ps[i][0:DH, :])
            pb = (j % 2) * 64
            nc.sync.dma_start(
                aot[pb:pb + DH, 2 * g + j // 2, :], ao_t[:])

    # software pipeline: attention of group g interleaves with the
    # k/v projection chunks (and setup) of group g+1, so the PE has
    # independent matmul work whenever it would stall on ACT exp.
    emit_group_setup(0)
    for ch in range(N // 512):
        emit_kv_chunk(0, ch)
    wo_c = []
    for g in range(G):
        if g + 1 < G:
            emit_group_setup(g + 1)
        else:
            # prefetch the o-proj weights into the (now idle) stream
            # slots while the last group's attention runs
            for oc in range(2):
                w = stream.tile([128, KT, 512], BF16, tag="wo",
                                name=f"wo_{oc}{R}")
                nc.gpsimd.dma_start(w[:], wop[:, oc])
                wo_c.append(w)
        for wave in range(2):
            pv_ps = [pvp.tile([DH + 1, M_SH], F32, tag="pv",
                              name=f"pv_{g}_{wave}_{i}{R}")
                     for i in range(2)]
            for nt in range(N // 128):
                if level >= 2:
                    emit_attn_nt(g, wave, nt, pv_ps)
                if g + 1 < G and nt % 8 == 7:
                    emit_kv_chunk(g + 1, 2 * wave + nt // 8)
            if level >= 2:
                emit_tails(g, wave, pv_ps)
        del gstate[g]

    # ---- output projection + bias + residual + LayerNorm ----
    for mt in range(M_SH // 128 if level >= 3 else 0):
        x_t = outst.tile([128, D], F32, tag="x")
        qn_t = outst.tile([128, D], F32, tag="qn")
        nc.scalar.dma_start(qn_t[:], qn[mt * 128:(mt + 1) * 128, :])
        for oc in range(2):
            ps = psp.tile([128, 512], F32, tag="mm")
            for kt in range(KT):
                nc.tensor.matmul(
                    ps[:], aot[:, kt, mt * 128:(mt + 1) * 128],
                    wo_c[oc][:, kt], start=(kt == 0), stop=(kt == KT - 1))
            nc.vector.tensor_add(out=x_t[:, oc * 512:(oc + 1) * 512],
                                 in0=ps[:],
                                 in1=bo_b[:, oc * 512:(oc + 1) * 512])
        nc.vector.tensor_add(out=x_t[:], in0=x_t[:], in1=qn_t[:])
        # LayerNorm over D=1024 (two bn_stats subgroups of 512)
        st = outst.tile([128, 2, 6], F32, tag="st")
        nc.vector.bn_stats(st[:, 0], x_t[:, 0:512])
        nc.vector.bn_stats(st[:, 1], x_t[:, 512:1024])
        mv = outst.tile([128, 2], F32, tag="mv")
        nc.vector.bn_aggr(mv[:], st[:])
        nm = outst.tile([128, 1], F32, tag="nm")
        nc.vector.tensor_scalar_mul(nm[:], mv[:, 0:1], -1.0)
        rstd = outst.tile([128, 1], F32, tag="rstd")
        nc.scalar.activation(out=rstd[:], in_=mv[:, 1:2],
                             func=AFT.Sqrt, bias=eps_sb[:], scale=1.0)
        nc.vector.reciprocal(rstd[:], rstd[:])
        nc.vector.tensor_scalar_add(x_t[:], x_t[:], nm[:])
        nc.vector.tensor_scalar_mul(x_t[:], x_t[:], rstd[:])
        nc.vector.tensor_mul(out=x_t[:], in0=x_t[:], in1=gamma_b[:])
        nc.vector.tensor_add(out=x_t[:], in0=x_t[:], in1=beta_b[:])
        nc.sync.dma_start(out[mt * 128:(mt + 1) * 128, :], x_t[:])


def _pack128(a_T, bfdt):
    """[D_in, X] contraction-major -> [128, D_in//128, X] partition-major."""
    ktl = a_T.shape[0] // 128
    return np.ascontiguousarray(
        a_T.reshape(ktl, 128, -1).transpose(1, 0, 2).astype(bfdt))


def make_in_maps(inputs):
    f = lambda x: np.ascontiguousarray(np.asarray(x, dtype=np.float32))
    bfdt = mybir.dt.np(BF16)
    Q, KV = f(inputs["Q"]), f(inputs["KV"])
    gate = f(inputs["log_gate_bias"])
    wq_T = f(np.asarray(inputs["Wq"]).T)   # [D_in, D_out]
    wk_T = f(np.asarray(inputs["Wk"]).T)
    wv_T = f(np.asarray(inputs["Wv"]).T)
    wo_T = f(np.asarray(inputs["Wo"]).T)

    # packed per-group weights: wq | wk | wv_augmented (ones col per head)
    wall = np.empty((128, G, WSEG), dtype=bfdt)
    for g in range(G):
        ob = g * OG
        wq_g = _pack128(wq_T[:, ob:ob + OG], bfdt).reshape(128, KT * OG)
        wk_g = _pack128(wk_T[:, ob:ob + OG], bfdt).reshape(128, KT * OG)
        wv_g = wv_T[:, ob:ob + OG].reshape(D, HPG, DH)
        wv_aug = np.zeros((D, HPG, DH + 1), np.float32)
        wv_aug[:, :, 0:DH] = wv_g
        wv_gp = _pack128(wv_aug.reshape(D, OGA), bfdt).reshape(128, KT * OGA)
        wall[:, g, 0:KT * OG] = wq_g
        wall[:, g, KT * OG:2 * KT * OG] = wk_g
        wall[:, g, 2 * KT * OG:] = wv_gp
    wop = _pack128(wo_T, bfdt).reshape(128, KT, 2, 512).transpose(
        0, 2, 1, 3)   # [128, oc, kt, 512]
    wop = np.ascontiguousarray(wop)

    bva = np.zeros((G, HPG, DH + 1), np.float32)
    bva[:, :, 0:DH] = f(inputs["bv"]).reshape(G, HPG, DH)
    bva[:, :, DH] = 1.0

    shared = {
        "wall": wall, "wop": wop,
        "bq": f(inputs["bq"]), "bk": f(inputs["bk"]),
        "bva": np.ascontiguousarray(bva.reshape(-1)),
        "bo": f(inputs["bo"]),
        "gamma": f(inputs["gamma"]), "beta": f(inputs["beta"]),
    }
    in_maps = []
    ck_cache = {}
    for c in range(N_CORES):
        b, mh = c // 2, c % 2
        if b not in ck_cache:
            ck_cache[b] = _pack128(KV[b].T, bfdt)
        in_maps.append({
            "qtp": _pack128(Q[b].T[:, mh * M_SH:(mh + 1) * M_SH], bfdt),
            "qn": f(Q[b, mh * M_SH:(mh + 1) * M_SH, :]),
            "ckp": ck_cache[b],
            "gate": np.ascontiguousarray(gate[b]),
            **shared,
        })
    return in_maps


def assemble(results):
    out = np.empty((B, M, D), dtype=np.float32)
    for c in range(N_CORES):
        b, mh = c // 2, c % 2
        out[b, mh * M_SH:(mh + 1) * M_SH, :] = results[c]["out"]
    return out


def kernel(**inputs) -> np.ndarray:
    if "nc" not in _CACHE:
        _CACHE["nc"] = build_nc()
    nc = _CACHE["nc"]
    in_maps = make_in_maps(inputs)
    res = run_bass_kernel_spmd(nc, in_maps, core_ids=list(range(N_CORES)))
    return assemble(res.results)
